# revision 45
# baseline (speedup 1.0000x reference)
"""Causal attention kernel for Trainium2, SPMD over 8 NeuronCores.

Problem: B=8, S=4096, D=128 fp32 causal attention
  scores = q @ k.T; logits = (scores - 1e9*triu1)/sqrt(128)
  out    = softmax(logits, axis=-1) @ v

Sharding: batch B=8 -> one batch element per core (data parallel), no
collectives.

Per-core v3 design (transposed-scores flash-style, no online softmax):
  - Q^T/K^T arrive host-transposed f32, loaded+cast to bf16 by gpsimd
    cast-DMAs (few coarse instrs; ~1us Pool descriptor time each). V bf16.
  - Scores computed TRANSPOSED and exactly causal, streamed tile-major
    (k-tile j covers q>=128j) into alternating 1536/1024-col PSUM chunks
    (2+2 double-buffer using 5 banks). The causal triangle is applied ON
    THE PE as a second accumulation pass closing the diagonal's PSUM group
    (lhsT=I128, rhs=mask with -60000 on k>q) so DVE/ACT never see mask
    work. NB: a start=True matmul into a bank wipes any OPEN group there —
    the mask pass must immediately follow its diagonal score pass.
  - ACT exp() reads full-width PSUM chunks (spanning tile seams; ~53
    instrs) and writes P^T to SBUF bf16 (ragged tile-major storage). ACT is
    the bottleneck engine: measured 0.80-0.95 ns/col + ~320ns/instr.
  - Denominators: running-sum buffer R[128, S] fp16 accumulated on DVE with
    one wide add per k-tile (mixed bf16+fp16 adds run in 2x mode). Group-
    closing tiles' head 128 cols (and all last-group tiles) skip R and are
    matmul'd from pt directly into the per-group ones-matmul partition
    reduction ([1,512] PSUM, 1 shared bank); tiny fp32 matmuls transpose
    the sums to partitions; reciprocal + scales on DVE.
  - PV: out^T[d,q] accumulated per 512-col q-group (2 PSUM banks), V_j
    stationary, P^T moving. Passes are emitted EAGERLY as their P^T slices
    are exp'd (3-chunk lag, <=4 passes per chunk to avoid starving ACT)
    with a 2-group open-accumulator window; closes are two-staged so PE
    never stalls on the DVE chain. Finalize: evict+transpose (xbar DMA),
    scale by 1/rowsum, cast-DMA out (last 2 groups: f32 + sync-queue DMA
    to keep the tail off the Pool queue).

Measured (loop-slope, unthrottled): ~99us/core vs baseline 125.6us; rel
err 3.76e-3. TimelineSim 92.2us. ACT busy ~81us is the roofline; PE ~56,
DVE ~50, Pool ~20. Sustained benching can trip the chip's power throttle
(reads 130-175us until it cools).
"""

import math
import sys

import numpy as np

try:
    import concourse.bass as bass
except ImportError:
    sys.path.insert(0, "/opt/trn_rl_repo")
    import concourse.bass as bass

import concourse.tile as tile
from concourse import bacc, mybir
from concourse.bass_utils import run_bass_kernel_spmd

D = 128
NCORES = 8
SCALE = 1.0 / math.sqrt(128.0)
NEG = -60000.0  # -60000*SCALE ~= -5300 -> exp == 0; fp16/bf16-safe magnitude
F32 = mybir.dt.float32
BF16 = mybir.dt.bfloat16
FP16 = mybir.dt.float16


def _build_mask() -> np.ndarray:
    """Triangle mask [128, 128] f32: m[k, q] = NEG where k > q (local)."""
    k = np.arange(128)[:, None]
    q = np.arange(128)[None, :]
    return np.where(k > q, np.float32(NEG), np.float32(0.0))


def _build_ident() -> np.ndarray:
    return np.eye(128, dtype=np.float32)


def build_attention_nc(S: int = 4096, chunkA: int = 1536, chunkB: int = 1024,
                       W: int = 512, loop_reps: int = 1, openw: int = 2,
                       debug_dump: bool = False):
    """Build the single-core Bass program (SPMD-replicated over cores)."""
    assert S % W == 0 and W % 128 == 0
    NT = S // 128          # k tiles
    NG = S // W            # q groups
    WB = W // 128          # 128-blocks per group
    TPG = NT // NG         # k-tiles per q-group (4)

    seglen = [S - 128 * j for j in range(NT)]
    off = [0]
    for j in range(NT):
        off.append(off[-1] + seglen[j])
    LTOT = off[NT]

    nc = bacc.Bacc("TRN2", target_bir_lowering=False, debug=False)

    qt_d = nc.declare_dram_parameter("qT", [128, S], F32, isOutput=False).ap()
    kt_d = nc.declare_dram_parameter("kT", [128, S], F32, isOutput=False).ap()
    v_d = nc.declare_dram_parameter("v", [S, D], F32, isOutput=False).ap()
    m_d = nc.declare_dram_parameter("mask", [128, 128], F32, isOutput=False).ap()
    i_d = nc.declare_dram_parameter("ident", [128, 128], F32, isOutput=False).ap()
    o_d = nc.declare_dram_parameter("out", [S, D], F32, isOutput=True).ap()

    v3 = v_d.rearrange("(t p) d -> p t d", p=128)
    o3 = o_d.rearrange("(g b p) d -> p g b d", p=128, b=WB)
    if debug_dump:
        ptd_d = nc.declare_dram_parameter("pt_dump", [128, LTOT], F32,
                                          isOutput=True).ap()
        rd_d = nc.declare_dram_parameter("r_dump", [128, S], F32,
                                         isOutput=True).ap()

    PC = 512  # input load chunk width

    with tile.TileContext(nc) as tc:
        with (
            tc.tile_pool(name="singles", bufs=1) as singles,
            tc.tile_pool(name="stage", bufs=2) as stage,
            tc.tile_pool(name="stpA", bufs=1, space="PSUM") as stpA,
            tc.tile_pool(name="stpB", bufs=1, space="PSUM") as stpB,
            tc.tile_pool(name="otp", bufs=2, space="PSUM") as otp,
            tc.tile_pool(name="clsp", bufs=1, space="PSUM") as clsp,
            tc.tile_pool(name="fin", bufs=2) as fin,
        ):
            # ---- persistent SBUF tensors ----
            qT = singles.tile([128, S], BF16, tag="qT")     # [d, q]
            kT = singles.tile([128, S], BF16, tag="kT")     # [d, k]
            vh = singles.tile([128, NT, 128], BF16, tag="vh")   # [k_loc, j, d]
            pt = singles.tile([128, LTOT], BF16, tag="pt")  # ragged P^T
            R = singles.tile([128, S], FP16, tag="R")       # running col-sums
            mskb = singles.tile([128, 128], BF16, tag="mskb")
            identb = singles.tile([128, 128], BF16, tag="identb")
            ones_h = singles.tile([128, 1], FP16, tag="ones")
            ones_b = singles.tile([128, 1], BF16, tag="onesb")
            one_el = singles.tile([1, 1], F32, tag="onel")

            nc.vector.memset(ones_h, 1.0)
            nc.vector.memset(ones_b, 1.0)
            nc.vector.memset(one_el, 1.0)

            # prologue cast-DMAs (gpsimd queue, in order of first use; few
            # coarse instrs — each one costs ~1us of Pool descriptor time)
            def ld(dst, src, lo, hi):
                lo, hi = min(lo, S), min(hi, S)
                if lo < hi:
                    nc.gpsimd.dma_start(out=dst[:, lo:hi], in_=src[:, lo:hi])

            # ident/mask via the empty sync queue + DVE cast (the Pool
            # DMA queue serializes ~1us per instr; these gate the 1st chunk)
            st_i = stage.tile([128, 128], F32, tag="stg")
            nc.sync.dma_start(out=st_i, in_=i_d)
            nc.vector.tensor_copy(out=identb, in_=st_i)
            st_m = stage.tile([128, 128], F32, tag="stg")
            nc.sync.dma_start(out=st_m, in_=m_d)
            nc.vector.tensor_copy(out=mskb, in_=st_m)
            ld(kT, kt_d, 0, 128)
            ld(qT, qt_d, 0, 512)
            ld(qT, qt_d, 512, 2 * chunkA)
            nc.gpsimd.dma_start(out=vh[:, 0:TPG, :], in_=v3[:, 0:TPG, :])
            ld(qT, qt_d, 2 * chunkA, S)
            ld(kT, kt_d, 128, 2048)
            if NT > TPG:
                nc.gpsimd.dma_start(out=vh[:, TPG:NT, :], in_=v3[:, TPG:NT, :])
            ld(kT, kt_d, 2048, S)

            def _emit_body():
                # ---------- bookkeeping ----------
                # DVE running-sum events: (ready_prefix, seq, kind, args).
                # Closer tiles (j%TPG==TPG-1) skip the head 128 cols: that
                # q-slice only feeds its own group's close, which reads pt
                # directly. Tiles of the LAST group (j>=NT-TPG) skip R
                # entirely for the same reason.
                dve_events = []
                for j in range(NT):
                    seg = seglen[j]
                    closer = (j % TPG == TPG - 1)
                    if j == 0:
                        dve_events.append((off[1], 0, "rcopy", j))
                    elif j >= NT - TPG:
                        continue
                    elif closer:
                        if seg > 128:
                            dve_events.append((off[j + 1], 1, "rbody", j))
                    else:
                        dve_events.append((off[j + 1], 0, "radd", j))
                dve_events.sort()
                dve_ptr = [0]

                # PV passes per group, ascending j (ready ascending)
                passes = []
                for g in range(NG):
                    lst = []
                    for j in range(TPG * (g + 1)):
                        ready = off[j] + W * (g + 1) - 128 * j
                        lst.append((ready, j))
                    passes.append(lst)
                pass_ptr = [0] * NG
                acc_tiles = [None] * NG
                ga_box = [0]
                pfx_hist = [0, 0, 0, 0]  # flush prefixes, ring of CLAG entries
                LAG = 3   # PV passes trail the sweep by LAG chunks
                CLAG = 4  # closes trail one more (R-add must clear DVE)

                def emit_dve(upto):
                    while dve_ptr[0] < len(dve_events):
                        ready, _, kind, j = dve_events[dve_ptr[0]]
                        if ready > upto:
                            break
                        seg = seglen[j]
                        if kind == "rcopy":
                            nc.vector.tensor_copy(out=R[:, 0:S], in_=pt[:, 0:S])
                        elif kind == "radd":
                            nc.vector.tensor_add(
                                out=R[:, 128 * j:S], in0=R[:, 128 * j:S],
                                in1=pt[:, off[j]:off[j] + seg])
                        elif kind == "rhead":
                            nc.vector.tensor_add(
                                out=R[:, 128 * j:128 * j + 128],
                                in0=R[:, 128 * j:128 * j + 128],
                                in1=pt[:, off[j]:off[j] + 128])
                        elif kind == "rbody":
                            nc.vector.tensor_add(
                                out=R[:, 128 * j + 128:S],
                                in0=R[:, 128 * j + 128:S],
                                in1=pt[:, off[j] + 128:off[j] + seg])
                        dve_ptr[0] += 1

                def emit_pass(g, j):
                    if acc_tiles[g] is None:
                        acc_tiles[g] = otp.tile([128, W], F32, tag="acc",
                                                name=f"acc{g % openw}")
                    qlo = max(W * g, 128 * j)
                    nc.tensor.matmul(
                        acc_tiles[g][:, qlo - W * g:W],
                        lhsT=vh[:, j, :],
                        rhs=pt[:, off[j] + qlo - 128 * j:off[j] + W * (g + 1) - 128 * j],
                        start=(j == 0),
                        stop=(j == TPG * (g + 1) - 1),
                        skip_group_check=True,
                    )

                stage2_q = []
                stage2b_q = []

                def emit_close1(g):
                    # stage 1: denominator partition-reduce + PSUM evictions
                    # (one shared PSUM bank: [1,W] sums then [128,WB] rs,
                    # temporally disjoint). Contributions not in R come from
                    # pt directly (closer head; all last-group tiles).
                    cls = clsp.tile([128, W], F32, tag="cls")
                    sums_ps = cls[0:1, :]
                    direct = ([NT - TPG + t for t in range(TPG)]
                              if g == NG - 1 else [TPG * (g + 1) - 1])
                    nc.tensor.matmul(sums_ps, lhsT=ones_h,
                                     rhs=R[:, W * g:W * (g + 1)],
                                     start=True, stop=False,
                                     skip_group_check=True)
                    for i, j in enumerate(direct):
                        qlo = max(W * g, 128 * j)
                        wid = (W * (g + 1) - qlo) if g == NG - 1 else 128
                        nc.tensor.matmul(
                            sums_ps[:, qlo - W * g:qlo - W * g + wid],
                            lhsT=ones_b,
                            rhs=pt[:, off[j] + qlo - 128 * j:
                                   off[j] + qlo - 128 * j + wid],
                            start=False, stop=(i == len(direct) - 1),
                            skip_group_check=True)
                    sums_s = fin.tile([1, W], F32, tag="sums")
                    nc.vector.tensor_copy(out=sums_s, in_=sums_ps)
                    ot_b = fin.tile([128, W], BF16, tag="otb")
                    nc.vector.tensor_copy(out=ot_b, in_=acc_tiles[g])
                    o_tr = fin.tile([128, WB, 128], BF16, tag="otr")
                    nc.sync.dma_start(out=o_tr, in_=ot_b, transpose=True)
                    stage2_q.append((g, cls, sums_s, o_tr))

                def emit_close2a():
                    # stage 2a (one drain later, so PE never stalls on DVE):
                    # transpose sums to partitions + reciprocal; frees cls
                    g, cls, sums_s, o_tr = stage2_q.pop(0)
                    rs_ps = cls[:, 0:WB]
                    for b in range(WB):
                        nc.tensor.matmul(
                            rs_ps[:, b:b + 1],
                            lhsT=sums_s[0:1, b * 128:(b + 1) * 128],
                            rhs=one_el, start=True, stop=True)
                    rinv = fin.tile([128, WB], F32, tag="rinv")
                    nc.vector.reciprocal(out=rinv, in_=rs_ps)
                    stage2b_q.append((g, rinv, o_tr))

                def emit_close2b():
                    # stage 2b: scale by 1/rowsum and DMA out (the scales
                    # wait on the xbar transpose; keep them last on DVE)
                    g, rinv, o_tr = stage2b_q.pop(0)
                    if g >= NG - 2:
                        # tail groups: scale to f32 and DMA out on the sync
                        # HWDGE queue (no Pool descriptor-gen on the tail)
                        o_f = fin.tile([128, WB, 128], F32, tag="of")
                        for b in range(WB):
                            nc.vector.tensor_scalar_mul(
                                out=o_f[:, b, :], in0=o_tr[:, b, :],
                                scalar1=rinv[:, b:b + 1])
                        nc.sync.dma_start(out=o3[:, g], in_=o_f)
                    else:
                        o_sc = fin.tile([128, WB, 128], BF16, tag="osc")
                        for b in range(WB):
                            nc.vector.tensor_scalar_mul(
                                out=o_sc[:, b, :], in0=o_tr[:, b, :],
                                scalar1=rinv[:, b:b + 1])
                        nc.gpsimd.dma_start(out=o3[:, g], in_=o_sc)

                def drain_pe(lag_pfx, close_pfx=None, final=False):
                    if close_pfx is None:
                        close_pfx = lag_pfx
                    # cap passes per drain: a newly-admitted group's backlog
                    # must not flood the PE queue ahead of the next chunk's
                    # scores (that starves ACT)
                    budget = [10 ** 9 if final else 4]

                    while stage2_q:
                        emit_close2a()
                    if stage2b_q:
                        emit_close2b()
                    while True:
                        ga = ga_box[0]
                        if ga >= NG:
                            return
                        grps = [g for g in range(ga, min(ga + openw, NG))]
                        for g in grps:
                            lst = passes[g]
                            while (budget[0] > 0 and pass_ptr[g] < len(lst)
                                   and (final or lst[pass_ptr[g]][0] <= lag_pfx)):
                                emit_pass(g, lst[pass_ptr[g]][1])
                                pass_ptr[g] += 1
                                budget[0] -= 1
                        if pass_ptr[ga] == len(passes[ga]) and (
                                final or passes[ga][-1][0] <= close_pfx):
                            # cls ring has one slot: retire the pending
                            # stage2a before the next close claims it
                            while stage2_q:
                                emit_close2a()
                            emit_close1(ga)
                            ga_box[0] += 1
                            continue
                        return

                # ---------- scores/exp sweep ----------
                st = {"filled": 0, "tile": None, "base": 0, "cap": 0, "idx": 0}

                def flush_chunk():
                    if st["tile"] is None:
                        return
                    w = st["filled"]
                    nc.scalar.activation(
                        out=pt[:, st["base"]:st["base"] + w],
                        in_=st["tile"][:, 0:w],
                        func=mybir.ActivationFunctionType.Exp,
                        scale=SCALE,
                    )
                    st["filled"] = 0
                    st["tile"] = None
                    prefix = st["base"] + w
                    emit_dve(prefix)
                    drain_pe(pfx_hist[-LAG], close_pfx=pfx_hist[-CLAG])
                    pfx_hist.append(prefix)
                    if len(pfx_hist) > CLAG:
                        pfx_hist.pop(0)

                for j in range(NT):
                    seg = seglen[j]
                    done = 0
                    while done < seg:
                        if st["tile"] is None:
                            even = st["idx"] % 2 == 0
                            cap = chunkA if even else chunkB
                            pool = stpA if even else stpB
                            st["tile"] = pool.tile([128, cap], F32, tag="st",
                                                   name="stx")
                            st["cap"] = cap
                            st["idx"] += 1
                            st["base"] = off[j] + done
                        o = st["filled"]
                        take = min(seg - done, st["cap"] - o)
                        p0 = 0
                        while p0 < take:  # split at psum banks + diag edge
                            col = done + p0
                            n = min(512 - (o + p0) % 512, take - p0)
                            if col < 128:
                                # diag: scores then mask as one closed
                                # accumulation group, before anything else
                                # start=True's this bank again
                                n = min(n, 128 - col)
                                nc.tensor.matmul(
                                    st["tile"][:, o + p0:o + p0 + n],
                                    lhsT=kT[:, j * 128:(j + 1) * 128],
                                    rhs=qT[:, 128 * j + col:128 * j + col + n],
                                    start=True, stop=False,
                                    skip_group_check=True,
                                )
                                nc.tensor.matmul(
                                    st["tile"][:, o + p0:o + p0 + n],
                                    lhsT=identb,
                                    rhs=mskb[:, col:col + n],
                                    start=False, stop=True,
                                    skip_group_check=True,
                                )
                            else:
                                nc.tensor.matmul(
                                    st["tile"][:, o + p0:o + p0 + n],
                                    lhsT=kT[:, j * 128:(j + 1) * 128],
                                    rhs=qT[:, 128 * j + col:128 * j + col + n],
                                    start=True, stop=True,
                                )
                            p0 += n
                        st["filled"] += take
                        done += take
                        if st["filled"] == st["cap"]:
                            flush_chunk()
                flush_chunk()
                emit_dve(LTOT)
                drain_pe(LTOT, final=True)
                while stage2_q:
                    emit_close2a()
                while stage2b_q:
                    emit_close2b()
                if debug_dump:
                    nc.gpsimd.dma_start(out=ptd_d, in_=pt)
                    nc.gpsimd.dma_start(out=rd_d, in_=R)

            if loop_reps > 1:
                with tc.For_i(0, loop_reps, 1) as _it:
                    _emit_body()
            else:
                _emit_body()

    nc.compile()
    return nc


_NC_CACHE: dict = {}


def _get_nc(S: int):
    if S not in _NC_CACHE:
        _NC_CACHE[S] = build_attention_nc(S)
    return _NC_CACHE[S]


def kernel(query: np.ndarray, keys: np.ndarray, values: np.ndarray) -> np.ndarray:
    B, S, d = query.shape
    assert d == D
    nc = _get_nc(S)
    mask = _build_mask()
    ident = _build_ident()
    in_maps = [
        {
            "qT": np.ascontiguousarray(query[b].T, dtype=np.float32),
            "kT": np.ascontiguousarray(keys[b].T, dtype=np.float32),
            "v": np.ascontiguousarray(values[b], dtype=np.float32),
            "mask": mask,
            "ident": ident,
        }
        for b in range(B)
    ]
    res = run_bass_kernel_spmd(nc, in_maps, core_ids=list(range(B)))
    return np.stack([res.results[b]["out"] for b in range(B)]).astype(np.float32)


if __name__ == "__main__":
    rng = np.random.default_rng(0)
    B, S = 8, 4096
    q = rng.standard_normal((B, S, D), dtype=np.float32)
    k = rng.standard_normal((B, S, D), dtype=np.float32)
    v = rng.standard_normal((B, S, D), dtype=np.float32)
    out = kernel(q, k, v)
    print(out.shape, out.dtype)


# revision 51
# speedup vs baseline: 1.0265x; 1.0265x over previous
"""Causal attention kernel for Trainium2, SPMD over 8 NeuronCores.

Problem: B=8, S=4096, D=128 fp32 causal attention
  scores = q @ k.T; logits = (scores - 1e9*triu1)/sqrt(128)
  out    = softmax(logits, axis=-1) @ v

Sharding: batch B=8 -> one batch element per core (data parallel), no
collectives.

Per-core v3 design (transposed-scores flash-style, no online softmax):
  - Q^T/K^T arrive host-transposed f32, loaded+cast to bf16 by gpsimd
    cast-DMAs (few coarse instrs; ~1us Pool descriptor time each). V bf16.
  - Scores computed TRANSPOSED and exactly causal, streamed tile-major
    (k-tile j covers q>=128j) into alternating 1536/1024-col PSUM chunks
    (2+2 double-buffer using 5 banks). The causal triangle is applied ON
    THE PE as a second accumulation pass closing the diagonal's PSUM group
    (lhsT=I128, rhs=mask with -60000 on k>q) so DVE/ACT never see mask
    work. NB: a start=True matmul into a bank wipes any OPEN group there —
    the mask pass must immediately follow its diagonal score pass.
  - ACT exp() reads full-width PSUM chunks (spanning tile seams; ~53
    instrs) and writes P^T to SBUF bf16 (ragged tile-major storage). ACT is
    the bottleneck engine: measured 0.80-0.95 ns/col + ~320ns/instr.
  - Denominators: running-sum buffer R[128, S] fp16 accumulated on DVE with
    one wide add per k-tile (mixed bf16+fp16 adds run in 2x mode). Group-
    closing tiles' head 128 cols (and all last-group tiles) skip R and are
    matmul'd from pt directly into the per-group ones-matmul partition
    reduction ([1,512] PSUM, 1 shared bank); tiny fp32 matmuls transpose
    the sums to partitions; reciprocal + scales on DVE.
  - PV: out^T[d,q] accumulated per 512-col q-group (2 PSUM banks), V_j
    stationary, P^T moving. Passes are emitted EAGERLY as their P^T slices
    are exp'd (3-chunk lag, <=4 passes per chunk to avoid starving ACT)
    with a 2-group open-accumulator window; closes are two-staged so PE
    never stalls on the DVE chain. Finalize: evict+transpose (xbar DMA),
    scale by 1/rowsum, cast-DMA out (last 2 groups: f32 + sync-queue DMA
    to keep the tail off the Pool queue).

Measured (loop-slope, unthrottled): ~99us/core vs baseline 125.6us; rel
err 3.76e-3. TimelineSim 92.2us. ACT busy ~81us is the roofline; PE ~56,
DVE ~50, Pool ~20. Sustained benching can trip the chip's power throttle
(reads 130-175us until it cools).
"""

import math
import sys

import numpy as np

try:
    import concourse.bass as bass
except ImportError:
    sys.path.insert(0, "/opt/trn_rl_repo")
    import concourse.bass as bass

import concourse.tile as tile
from concourse import bacc, mybir
from concourse.bass_utils import run_bass_kernel_spmd

D = 128
NCORES = 8
SCALE = 1.0 / math.sqrt(128.0)
NEG = -60000.0  # -60000*SCALE ~= -5300 -> exp == 0; fp16/bf16-safe magnitude
F32 = mybir.dt.float32
BF16 = mybir.dt.bfloat16
FP16 = mybir.dt.float16


def _build_mask() -> np.ndarray:
    """Triangle mask [128, 128] f32: m[k, q] = 0 where k > q (local), else 1.
    Multiplied into exp(scores) on DVE after the fact."""
    k = np.arange(128)[:, None]
    q = np.arange(128)[None, :]
    return np.where(k > q, np.float32(0.0), np.float32(1.0))


def _build_ident() -> np.ndarray:
    return np.eye(128, dtype=np.float32)


def build_attention_nc(S: int = 4096, chunkA: int = 1536, chunkB: int = 1024,
                       W: int = 512, loop_reps: int = 1, openw: int = 2,
                       debug_dump: bool = False):
    """Build the single-core Bass program (SPMD-replicated over cores)."""
    assert S % W == 0 and W % 128 == 0
    NT = S // 128          # k tiles
    NG = S // W            # q groups
    WB = W // 128          # 128-blocks per group
    TPG = NT // NG         # k-tiles per q-group (4)

    seglen = [S - 128 * j for j in range(NT)]
    off = [0]
    for j in range(NT):
        off.append(off[-1] + seglen[j])
    LTOT = off[NT]

    nc = bacc.Bacc("TRN2", target_bir_lowering=False, debug=False)

    qt_d = nc.declare_dram_parameter("qT", [128, S], F32, isOutput=False).ap()
    kt_d = nc.declare_dram_parameter("kT", [128, S], F32, isOutput=False).ap()
    v_d = nc.declare_dram_parameter("v", [S, D], F32, isOutput=False).ap()
    m_d = nc.declare_dram_parameter("mask", [128, 128], F32, isOutput=False).ap()
    i_d = nc.declare_dram_parameter("ident", [128, 128], F32, isOutput=False).ap()
    o_d = nc.declare_dram_parameter("out", [S, D], F32, isOutput=True).ap()

    v3 = v_d.rearrange("(t p) d -> p t d", p=128)
    o3 = o_d.rearrange("(g b p) d -> p g b d", p=128, b=WB)
    if debug_dump:
        ptd_d = nc.declare_dram_parameter("pt_dump", [128, LTOT], F32,
                                          isOutput=True).ap()
        rd_d = nc.declare_dram_parameter("r_dump", [128, S], F32,
                                         isOutput=True).ap()

    PC = 512  # input load chunk width

    with tile.TileContext(nc) as tc:
        with (
            tc.tile_pool(name="singles", bufs=1) as singles,
            tc.tile_pool(name="stage", bufs=2) as stage,
            tc.tile_pool(name="stpA", bufs=1, space="PSUM") as stpA,
            tc.tile_pool(name="stpB", bufs=1, space="PSUM") as stpB,
            tc.tile_pool(name="otp", bufs=2, space="PSUM") as otp,
            tc.tile_pool(name="clsp", bufs=1, space="PSUM") as clsp,
            tc.tile_pool(name="fin", bufs=2) as fin,
        ):
            # ---- persistent SBUF tensors ----
            qT = singles.tile([128, S], BF16, tag="qT")     # [d, q]
            kT = singles.tile([128, S], BF16, tag="kT")     # [d, k]
            vh = singles.tile([128, NT, 128], BF16, tag="vh")   # [k_loc, j, d]
            pt = singles.tile([128, LTOT], BF16, tag="pt")  # ragged P^T
            R = singles.tile([128, S], FP16, tag="R")       # running col-sums
            mskb = singles.tile([128, 128], BF16, tag="mskb")
            ones_h = singles.tile([128, 1], FP16, tag="ones")
            ones_b = singles.tile([128, 1], BF16, tag="onesb")
            one_el = singles.tile([1, 1], F32, tag="onel")

            nc.vector.memset(ones_h, 1.0)
            nc.vector.memset(ones_b, 1.0)
            nc.vector.memset(one_el, 1.0)

            # prologue cast-DMAs (gpsimd queue, in order of first use; few
            # coarse instrs — each one costs ~1us of Pool descriptor time)
            def ld(dst, src, lo, hi):
                lo, hi = min(lo, S), min(hi, S)
                if lo < hi:
                    nc.gpsimd.dma_start(out=dst[:, lo:hi], in_=src[:, lo:hi])

            # mask via the empty sync queue + DVE cast (the Pool DMA queue
            # serializes ~1us per instr; this gates the 1st chunk's masking)
            st_m = stage.tile([128, 128], F32, tag="stg")
            nc.sync.dma_start(out=st_m, in_=m_d)
            nc.vector.tensor_copy(out=mskb, in_=st_m)
            ld(kT, kt_d, 0, 128)
            ld(qT, qt_d, 0, 512)
            ld(qT, qt_d, 512, 2 * chunkA)
            nc.gpsimd.dma_start(out=vh[:, 0:TPG, :], in_=v3[:, 0:TPG, :])
            ld(qT, qt_d, 2 * chunkA, S)
            ld(kT, kt_d, 128, 2048)
            if NT > TPG:
                nc.gpsimd.dma_start(out=vh[:, TPG:NT, :], in_=v3[:, TPG:NT, :])
            ld(kT, kt_d, 2048, S)

            def _emit_body():
                # ---------- bookkeeping ----------
                # DVE running-sum events: (ready_prefix, seq, kind, args).
                # Closer tiles (j%TPG==TPG-1) skip the head 128 cols: that
                # q-slice only feeds its own group's close, which reads pt
                # directly. Tiles of the LAST group (j>=NT-TPG) skip R
                # entirely for the same reason.
                dve_events = []
                for j in range(NT):
                    seg = seglen[j]
                    closer = (j % TPG == TPG - 1)
                    # causal 0/1 triangle multiplied into the diag block
                    # right after its exp (before any consumer)
                    dve_events.append((off[j] + 128, -1, "mask", j))
                    if j == 0:
                        dve_events.append((off[1], 0, "rcopy", j))
                    elif j >= NT - TPG:
                        continue
                    elif closer:
                        if seg > 128:
                            dve_events.append((off[j + 1], 1, "rbody", j))
                    else:
                        dve_events.append((off[j + 1], 0, "radd", j))
                dve_events.sort()
                dve_ptr = [0]

                # PV passes per group, ascending j (ready ascending)
                passes = []
                for g in range(NG):
                    lst = []
                    for j in range(TPG * (g + 1)):
                        ready = off[j] + W * (g + 1) - 128 * j
                        lst.append((ready, j))
                    passes.append(lst)
                pass_ptr = [0] * NG
                acc_tiles = [None] * NG
                ga_box = [0]
                pfx_hist = [0, 0, 0, 0]  # flush prefixes, ring of CLAG entries
                LAG = 3   # PV passes trail the sweep by LAG chunks
                CLAG = 4  # closes trail one more (R-add must clear DVE)

                def emit_dve(upto):
                    while dve_ptr[0] < len(dve_events):
                        ready, _, kind, j = dve_events[dve_ptr[0]]
                        if ready > upto:
                            break
                        seg = seglen[j]
                        if kind == "mask":
                            nc.vector.tensor_mul(
                                out=pt[:, off[j]:off[j] + 128],
                                in0=pt[:, off[j]:off[j] + 128],
                                in1=mskb)
                        elif kind == "rcopy":
                            nc.vector.tensor_copy(out=R[:, 0:S], in_=pt[:, 0:S])
                        elif kind == "radd":
                            nc.vector.tensor_add(
                                out=R[:, 128 * j:S], in0=R[:, 128 * j:S],
                                in1=pt[:, off[j]:off[j] + seg])
                        elif kind == "rhead":
                            nc.vector.tensor_add(
                                out=R[:, 128 * j:128 * j + 128],
                                in0=R[:, 128 * j:128 * j + 128],
                                in1=pt[:, off[j]:off[j] + 128])
                        elif kind == "rbody":
                            nc.vector.tensor_add(
                                out=R[:, 128 * j + 128:S],
                                in0=R[:, 128 * j + 128:S],
                                in1=pt[:, off[j] + 128:off[j] + seg])
                        dve_ptr[0] += 1

                def emit_pass(g, j):
                    if acc_tiles[g] is None:
                        acc_tiles[g] = otp.tile([128, W], F32, tag="acc",
                                                name=f"acc{g % openw}")
                    qlo = max(W * g, 128 * j)
                    nc.tensor.matmul(
                        acc_tiles[g][:, qlo - W * g:W],
                        lhsT=vh[:, j, :],
                        rhs=pt[:, off[j] + qlo - 128 * j:off[j] + W * (g + 1) - 128 * j],
                        start=(j == 0),
                        stop=(j == TPG * (g + 1) - 1),
                        skip_group_check=True,
                    )

                stage2_q = []
                stage2b_q = []

                def emit_close1(g):
                    # stage 1: denominator partition-reduce + PSUM evictions
                    # (one shared PSUM bank: [1,W] sums then [128,WB] rs,
                    # temporally disjoint). Contributions not in R come from
                    # pt directly (closer head; all last-group tiles).
                    cls = clsp.tile([128, W], F32, tag="cls")
                    sums_ps = cls[0:1, :]
                    direct = ([NT - TPG + t for t in range(TPG)]
                              if g == NG - 1 else [TPG * (g + 1) - 1])
                    nc.tensor.matmul(sums_ps, lhsT=ones_h,
                                     rhs=R[:, W * g:W * (g + 1)],
                                     start=True, stop=False,
                                     skip_group_check=True)
                    for i, j in enumerate(direct):
                        qlo = max(W * g, 128 * j)
                        wid = (W * (g + 1) - qlo) if g == NG - 1 else 128
                        nc.tensor.matmul(
                            sums_ps[:, qlo - W * g:qlo - W * g + wid],
                            lhsT=ones_b,
                            rhs=pt[:, off[j] + qlo - 128 * j:
                                   off[j] + qlo - 128 * j + wid],
                            start=False, stop=(i == len(direct) - 1),
                            skip_group_check=True)
                    sums_s = fin.tile([1, W], F32, tag="sums")
                    nc.vector.tensor_copy(out=sums_s, in_=sums_ps)
                    ot_b = fin.tile([128, W], BF16, tag="otb")
                    nc.vector.tensor_copy(out=ot_b, in_=acc_tiles[g])
                    o_tr = fin.tile([128, WB, 128], BF16, tag="otr")
                    nc.sync.dma_start(out=o_tr, in_=ot_b, transpose=True)
                    stage2_q.append((g, cls, sums_s, o_tr))

                def emit_close2a():
                    # stage 2a (one drain later, so PE never stalls on DVE):
                    # transpose sums to partitions + reciprocal; frees cls
                    g, cls, sums_s, o_tr = stage2_q.pop(0)
                    rs_ps = cls[:, 0:WB]
                    for b in range(WB):
                        nc.tensor.matmul(
                            rs_ps[:, b:b + 1],
                            lhsT=sums_s[0:1, b * 128:(b + 1) * 128],
                            rhs=one_el, start=True, stop=True)
                    rinv = fin.tile([128, WB], F32, tag="rinv")
                    nc.vector.reciprocal(out=rinv, in_=rs_ps)
                    stage2b_q.append((g, rinv, o_tr))

                def emit_close2b():
                    # stage 2b: scale by 1/rowsum and DMA out (the scales
                    # wait on the xbar transpose; keep them last on DVE)
                    g, rinv, o_tr = stage2b_q.pop(0)
                    if g >= NG - 2:
                        # tail groups: scale to f32 and DMA out on the sync
                        # HWDGE queue (no Pool descriptor-gen on the tail)
                        o_f = fin.tile([128, WB, 128], F32, tag="of")
                        for b in range(WB):
                            nc.vector.tensor_scalar_mul(
                                out=o_f[:, b, :], in0=o_tr[:, b, :],
                                scalar1=rinv[:, b:b + 1])
                        nc.sync.dma_start(out=o3[:, g], in_=o_f)
                    else:
                        o_sc = fin.tile([128, WB, 128], BF16, tag="osc")
                        for b in range(WB):
                            nc.vector.tensor_scalar_mul(
                                out=o_sc[:, b, :], in0=o_tr[:, b, :],
                                scalar1=rinv[:, b:b + 1])
                        nc.gpsimd.dma_start(out=o3[:, g], in_=o_sc)

                def drain_pe(lag_pfx, close_pfx=None, final=False):
                    if close_pfx is None:
                        close_pfx = lag_pfx
                    # cap passes per drain: a newly-admitted group's backlog
                    # must not flood the PE queue ahead of the next chunk's
                    # scores (that starves ACT)
                    budget = [10 ** 9 if final else 4]

                    while stage2_q:
                        emit_close2a()
                    if stage2b_q:
                        emit_close2b()
                    while True:
                        ga = ga_box[0]
                        if ga >= NG:
                            return
                        grps = [g for g in range(ga, min(ga + openw, NG))]
                        for g in grps:
                            lst = passes[g]
                            while (budget[0] > 0 and pass_ptr[g] < len(lst)
                                   and (final or lst[pass_ptr[g]][0] <= lag_pfx)):
                                emit_pass(g, lst[pass_ptr[g]][1])
                                pass_ptr[g] += 1
                                budget[0] -= 1
                        if pass_ptr[ga] == len(passes[ga]) and (
                                final or passes[ga][-1][0] <= close_pfx):
                            # cls ring has one slot: retire the pending
                            # stage2a before the next close claims it
                            while stage2_q:
                                emit_close2a()
                            emit_close1(ga)
                            ga_box[0] += 1
                            continue
                        return

                # ---------- scores/exp sweep ----------
                st = {"filled": 0, "tile": None, "base": 0, "cap": 0, "idx": 0}

                def flush_chunk():
                    if st["tile"] is None:
                        return
                    w = st["filled"]
                    nc.scalar.activation(
                        out=pt[:, st["base"]:st["base"] + w],
                        in_=st["tile"][:, 0:w],
                        func=mybir.ActivationFunctionType.Exp,
                        scale=SCALE,
                    )
                    st["filled"] = 0
                    st["tile"] = None
                    prefix = st["base"] + w
                    emit_dve(prefix)
                    drain_pe(pfx_hist[-LAG], close_pfx=pfx_hist[-CLAG])
                    pfx_hist.append(prefix)
                    if len(pfx_hist) > CLAG:
                        pfx_hist.pop(0)

                for j in range(NT):
                    seg = seglen[j]
                    done = 0
                    while done < seg:
                        if st["tile"] is None:
                            even = st["idx"] % 2 == 0
                            cap = chunkA if even else chunkB
                            pool = stpA if even else stpB
                            st["tile"] = pool.tile([128, cap], F32, tag="st",
                                                   name="stx")
                            st["cap"] = cap
                            st["idx"] += 1
                            st["base"] = off[j] + done
                        o = st["filled"]
                        take = min(seg - done, st["cap"] - o)
                        p0 = 0
                        while p0 < take:  # split at psum bank boundaries
                            col = done + p0
                            n = min(512 - (o + p0) % 512, take - p0)
                            nc.tensor.matmul(
                                st["tile"][:, o + p0:o + p0 + n],
                                lhsT=kT[:, j * 128:(j + 1) * 128],
                                rhs=qT[:, 128 * j + col:128 * j + col + n],
                                start=True, stop=True,
                            )
                            p0 += n
                        st["filled"] += take
                        done += take
                        if st["filled"] == st["cap"]:
                            flush_chunk()
                flush_chunk()
                emit_dve(LTOT)
                drain_pe(LTOT, final=True)
                while stage2_q:
                    emit_close2a()
                while stage2b_q:
                    emit_close2b()
                if debug_dump:
                    nc.gpsimd.dma_start(out=ptd_d, in_=pt)
                    nc.gpsimd.dma_start(out=rd_d, in_=R)

            if loop_reps > 1:
                with tc.For_i(0, loop_reps, 1) as _it:
                    _emit_body()
            else:
                _emit_body()

    nc.compile()
    return nc


_NC_CACHE: dict = {}


def _get_nc(S: int):
    if S not in _NC_CACHE:
        _NC_CACHE[S] = build_attention_nc(S)
    return _NC_CACHE[S]


def kernel(query: np.ndarray, keys: np.ndarray, values: np.ndarray) -> np.ndarray:
    B, S, d = query.shape
    assert d == D
    nc = _get_nc(S)
    mask = _build_mask()
    ident = _build_ident()
    in_maps = [
        {
            "qT": np.ascontiguousarray(query[b].T, dtype=np.float32),
            "kT": np.ascontiguousarray(keys[b].T, dtype=np.float32),
            "v": np.ascontiguousarray(values[b], dtype=np.float32),
            "mask": mask,
            "ident": ident,
        }
        for b in range(B)
    ]
    res = run_bass_kernel_spmd(nc, in_maps, core_ids=list(range(B)))
    return np.stack([res.results[b]["out"] for b in range(B)]).astype(np.float32)


if __name__ == "__main__":
    rng = np.random.default_rng(0)
    B, S = 8, 4096
    q = rng.standard_normal((B, S, D), dtype=np.float32)
    k = rng.standard_normal((B, S, D), dtype=np.float32)
    v = rng.standard_normal((B, S, D), dtype=np.float32)
    out = kernel(q, k, v)
    print(out.shape, out.dtype)
